# revision 41
# baseline (speedup 1.0000x reference)
"""Graph-ODE (GCN message passing) Trainium2 kernel.

Problem: h0 = x @ W_fc + b_fc; 4 Euler steps of
  h <- h + 0.25 * relu(gcn2(relu(gcn1(h)))),  gcn(h) = (adj @ h) @ W + b
with B=32, N=4096, IN_DIM=64, H=128.

Strategy (8 NeuronCores, data-parallel over batch):
 - Each core owns 4 batches; adj (pre-transposed + tiled on host) and
   weights are replicated. No collectives.
 - Aggregation adj @ V runs with the ACTIVATION as the stationary
   operand (V slab [m,128h] per batch) and adjT as the moving operand
   ([m, 512n] chunks), in fp8-e4m3 DoubleRow (K=256/slab). The output
   lands PRE-TRANSPOSED [h, n] in PSUM, so the projection consumes it
   directly (stationary=aggT slab, moving=W) and emits z back in
   [n, h] node-major form — the form the next aggregation's stationary
   needs. No PE transposes anywhere (the old moving-V scheme burned
   ~1024 transposes + ACT drains per run).
 - adj is scaled by 4096 on the host so entries sit in e4m3 normal
   range; the scale folds back via W/4096 in the projection.
 - Step-0 layer-1 aggregates x directly (adj@(x@Wfc) = (adj@x)@Wfc with
   W_fc@W1 folded on the host); two batches share one stationary
   ([m, 2x64] columns) so the 64-wide features still halve that layer.
 - Projections for unit k are emitted between the aggregation chains of
   unit k+1 so the PE never waits on the PSUM->SBUF drain.
 - Euler state h stays fp32 in SBUF; h0 = x@W_fc uses a 3-term bf16
   hi/lo split since it dominates the output.
 - DMA: adjT streams as 2MB chunks split across the sync (HW-DGE) and
   gpsimd (software-DGE) queues; phase-0 x rides sync/scalar. Outputs
   leave per-unit as single contiguous-per-partition DMAs (h state and
   the out tensor are p-major; host transposes back) because software
   DGE triggers cost ~640ns of gpsimd engine time each. Tail outputs
   ride sync so the kernel doesn't end on a software-queue drain.
   Measured: ~955-970us HW exec (PE ~95% busy; the fp8-DoubleRow
   aggregation streams at its 1-row/cycle silicon rate, ~829us of the
   ~908us total PE work), 6.9e-5 relative error vs the fp32 reference.
   (Prior moving-V scheme with PE transposes: 1270us on the same HW —
   its strided moving-operand APs also held the PE ~20% under rate.)
"""
import sys

sys.path.insert(0, "/opt/trn_rl_repo")

import numpy as np
import ml_dtypes

import concourse.bass as bass
import concourse.mybir as mybir
import concourse.tile as tile
from concourse.bass_utils import run_bass_kernel_spmd

BF16 = mybir.dt.bfloat16
FP8 = mybir.dt.float8e4
F32 = mybir.dt.float32
ADJ_SCALE = 4096.0

B, N, IN_DIM, H = 32, 4096, 64, 128
N_CORES = 8
BL = B // N_CORES          # 4 batches per core
NT = N // 128              # 32 node tiles
NCH = 8                    # 512-wide n chunks
CHW = N // NCH             # 512
STEP = 0.25
N_STEPS = 4


def _split_multiwait(nc):
    """This walrus build accepts only ONE sync-wait command per engine
    instruction (incl. drains). Hoist extra waits onto preceding
    single-wait InstNoOps on the same engine."""
    import bass_rust
    for fn in nc.m.functions:
        for blk in fn.blocks:
            out = []
            for inst in blk.instructions:
                si = inst.sync_info
                if (si is not None and si.on_wait and len(si.on_wait) > 1
                        and type(inst).__name__ not in (
                            "InstTensorLoad", "InstTensorSave", "InstTrigger")):
                    waits = list(si.on_wait)
                    for w in waits[:-1]:
                        out.append(mybir.InstNoOp(
                            name=nc.get_next_instruction_name(),
                            engine=inst.engine, ins=[], outs=[],
                            sync_info=bass_rust.SyncInfo(
                                on_wait=[w], on_update=[]),
                        ))
                    inst.sync_info = bass_rust.SyncInfo(
                        on_wait=[waits[-1]], on_update=list(si.on_update))
                out.append(inst)
            blk.instructions = out


def _build(with_bias):
    nc = bass.Bass()
    x_folded = not with_bias

    # adjT chunked: [chunk, p, mt2, i, n'] with m=(2*mt2+i)*128+p,
    # n = chunk*512+n'; scaled by ADJ_SCALE, fp8.
    adjm = nc.dram_tensor("adjm", [NCH, 128, NT // 2, 2, CHW], FP8,
                          kind="ExternalInput")
    if x_folded:
        x8 = nc.dram_tensor("x8", [128, NT, 2, 128], FP8, kind="ExternalInput")
        wfc1s = nc.dram_tensor("wfc1s", [128, H], BF16, kind="ExternalInput")
    xt_hi = nc.dram_tensor("xt_hi", [BL, IN_DIM, N], BF16, kind="ExternalInput")
    xt_lo = nc.dram_tensor("xt_lo", [BL, IN_DIM, N], BF16, kind="ExternalInput")
    wpack = nc.dram_tensor("wpack", [128, 512], BF16, kind="ExternalInput")
    if with_bias:
        b_fc = nc.dram_tensor("b_fc", [1, H], BF16, kind="ExternalInput")
        b1 = nc.dram_tensor("b1", [1, H], BF16, kind="ExternalInput")
        b2 = nc.dram_tensor("b2", [1, H], BF16, kind="ExternalInput")
        ones = nc.dram_tensor("ones", [1, H], BF16, kind="ExternalInput")
    out = nc.dram_tensor("out", [BL, 128, NT, H], F32, kind="ExternalOutput")

    relu = mybir.ActivationFunctionType.Relu
    DR = mybir.MatmulPerfMode.DoubleRow

    with tile.TileContext(nc) as tc:
        with tc.tile_pool(name="res", bufs=1) as res, \
             tc.tile_pool(name="wgt", bufs=1) as wgt, \
             tc.tile_pool(name="xs", bufs=3) as xs, \
             tc.tile_pool(name="adjs", bufs=4) as adjs, \
             tc.tile_pool(name="work", bufs=3) as work, \
             tc.tile_pool(name="psA", bufs=3, space="PSUM") as psA, \
             tc.tile_pool(name="psZ", bufs=3, space="PSUM") as psZ:

            # --- resident state: fp8 activations in [p, mt, b, h]
            # node-major form (node m = mt*128 + p) to serve as matmul
            # stationaries; fp32 h in [p, b, mt, h] so each unit's final
            # output is one contiguous-per-partition DMA
            Hsb = res.tile([128, BL, NT, H], F32, tag="Hsb")
            Hbf = res.tile([128, NT, BL, H], FP8, tag="Hbf")
            Tbf = res.tile([128, NT, BL, H], FP8, tag="Tbf")

            # --- constants
            wpack_t = wgt.tile([128, 512], BF16, tag="wpack")
            nc.sync.dma_start(wpack_t[:], wpack[:])
            w1_t = wpack_t[:, 0:128]
            w2_t = wpack_t[:, 128:256]
            wfc_hi_t = wpack_t[0:IN_DIM, 256:384]
            wfc_lo_t = wpack_t[0:IN_DIM, 384:512]
            wfc_hi2_t = wpack_t[IN_DIM:128, 256:384]
            if x_folded:
                wfc1s_t = wgt.tile([128, H], BF16, tag="wfc1s")
                nc.sync.dma_start(wfc1s_t[:], wfc1s[:])
            if with_bias:
                bfc_t = wgt.tile([1, H], BF16, tag="bfc")
                b1_t = wgt.tile([1, H], BF16, tag="b1")
                b2_t = wgt.tile([1, H], BF16, tag="b2")
                ones_t = wgt.tile([1, H], BF16, tag="ones")
                nc.sync.dma_start(bfc_t[:], b_fc[:])
                nc.sync.dma_start(b1_t[:], b1[:])
                nc.sync.dma_start(b2_t[:], b2[:])
                nc.sync.dma_start(ones_t[:], ones[:])

            # --- phase 0: h0 = x @ W_fc (+ b_fc), 3-term bf16 hi/lo split.
            # One unit = one (batch, 512-col chunk): 4 nt tiles into one
            # PSUM bank, drained with one wide vector copy.
            def emit_p0_unit(b, c):
                # xh on partitions 0-63 (array tile T0), xl on 64-127 (T8):
                # the 64x128 row-tiled matmuls overlap in the PE array, so
                # the xl@W_hi term rides free next to the two xh terms.
                x2 = xs.tile([128, CHW], BF16, tag="x2")
                nc.sync.dma_start(x2[0:IN_DIM, :], xt_hi[b, :, bass.ts(c, CHW)])
                nc.scalar.dma_start(x2[IN_DIM:128, :],
                                    xt_lo[b, :, bass.ts(c, CHW)])
                pz = psZ.tile([128, 4, H], F32, tag="pz")
                pzB = psZ.tile([128, 4, H], F32, tag="pzB", bufs=2)
                for j in range(4):
                    xhs = x2[0:IN_DIM, bass.ts(j, 128)]
                    xls = x2[IN_DIM:128, bass.ts(j, 128)]
                    nc.tensor.matmul(pz[:, j, :], xhs, wfc_hi_t,
                                     start=True, stop=False)
                    nc.tensor.matmul(pz[:, j, :], xhs, wfc_lo_t,
                                     start=False, stop=not with_bias)
                    if with_bias:
                        nc.tensor.matmul(pz[:, j, :], ones_t[:], bfc_t[:],
                                         start=False, stop=True)
                    nc.tensor.matmul(pzB[:, j, :], xls, wfc_hi2_t,
                                     start=True, stop=True)
                nc.vector.tensor_copy(Hsb[:, b, bass.ts(c, 4), :], pz[:])
                nc.vector.tensor_add(Hsb[:, b, bass.ts(c, 4), :],
                                     Hsb[:, b, bass.ts(c, 4), :], pzB[:])
                if not x_folded:
                    nc.scalar.activation(
                        Hbf[:, bass.ts(c, 4), b, :],
                        Hsb[:, b, bass.ts(c, 4), :],
                        mybir.ActivationFunctionType.Copy)

            # phase-0 units: a few emitted up front to cover the initial
            # adjT/x8 fills, the rest interleaved between x-layer units
            p0_iter = iter([(b, c) for b in range(BL) for c in range(NCH)])

            def emit_some_p0(n):
                for _ in range(n):
                    u = next(p0_iter, None)
                    if u is not None:
                        emit_p0_unit(*u)

            emit_some_p0(8 if x_folded else 32)

            if x_folded:
                X8t = res.tile([128, NT, 2, 128], FP8, tag="X8t")
                for c8 in range(4):
                    nc.gpsimd.dma_start(X8t[:, bass.ts(c8, 8), :, :],
                                        x8[:, bass.ts(c8, 8), :, :])

            # --- adjT chunk streaming (re-DMAed per aggregation pass);
            # each 2MB chunk split across the sync (HW-DGE) and gpsimd
            # (software-DGE) queues
            def load_chunk(c):
                ch = adjs.tile([128, NT // 2, 2, CHW], FP8, tag="chunk")
                nc.sync.dma_start(ch[:, 0:NT // 4, :, :], adjm[c, :, 0:NT // 4])
                nc.gpsimd.dma_start(ch[:, NT // 4:, :, :], adjm[c, :, NT // 4:])
                return ch

            # Deferred-projection queue: the proj/drain of unit k is
            # emitted after the aggregation chains of unit k+1 so the
            # PE never waits on the PSUM->SBUF drain.
            pending = [None]

            def flush_pending():
                if pending[0] is not None:
                    pending[0]()
                    pending[0] = None

            # --- step0/layer1 via x: adj@(x@Wfc) = (adj@x)@Wfc with
            # W_fc@W1 folded on the host. Stationary = x slab
            # [m, 2 batches x 64 feats] so 64-wide features halve it.
            if x_folded:
                for c in range(NCH):
                    ch = load_chunk(c)
                    for bpair in range(2):
                        pa = psA.tile([128, CHW], F32, tag="pa")
                        for mt2 in range(NT // 2):
                            nc.tensor.matmul(
                                pa[:], X8t[:, bass.ts(mt2, 2), bpair, :],
                                ch[:, mt2, :, :],
                                start=(mt2 == 0), stop=(mt2 == NT // 2 - 1),
                                perf_mode=DR)

                        def xproj(c=c, bpair=bpair, pa=pa):
                            ag = work.tile([128, CHW], BF16, tag="ag")
                            nc.vector.tensor_copy(ag[:], pa[:])
                            for bp in range(2):
                                b = 2 * bpair + bp
                                pz = psZ.tile([128, 4, H], F32, tag="pz")
                                for s in range(4):
                                    nc.tensor.matmul(
                                        pz[:, s, :],
                                        ag[bass.ds(64 * bp, 64),
                                           bass.ts(s, 128)],
                                        wfc1s_t[bass.ds(64 * bp, 64), :],
                                        start=True, stop=True)
                                nc.scalar.activation(
                                    Tbf[:, bass.ts(c, 4), b, :], pz[:], relu)

                        flush_pending()
                        pending[0] = xproj
                        emit_some_p0(2)
                emit_some_p0(32)

            # --- 4 Euler steps x 2 GCN layers ---
            for step in range(N_STEPS):
                for layer in range(2):
                    if x_folded and step == 0 and layer == 0:
                        continue
                    V = Hbf if layer == 0 else Tbf
                    W = w1_t if layer == 0 else w2_t
                    bias = None if not with_bias else (b1_t if layer == 0 else b2_t)
                    for c in range(NCH):
                        ch = load_chunk(c)
                        for b in range(BL):
                            pa = psA.tile([128, CHW], F32, tag="pa")
                            for mt2 in range(NT // 2):
                                nc.tensor.matmul(
                                    pa[:], V[:, bass.ts(mt2, 2), b, :],
                                    ch[:, mt2, :, :],
                                    start=(mt2 == 0), stop=(mt2 == NT // 2 - 1),
                                    perf_mode=DR)

                            def proj(c=c, b=b, pa=pa, W=W, bias=bias,
                                     layer=layer, step=step):
                                ag = work.tile([128, CHW], BF16, tag="ag")
                                nc.vector.tensor_copy(ag[:], pa[:])
                                pz = psZ.tile([128, 4, H], F32, tag="pz")
                                for s in range(4):
                                    nc.tensor.matmul(
                                        pz[:, s, :], ag[:, bass.ts(s, 128)], W,
                                        start=True, stop=bias is None)
                                    if bias is not None:
                                        nc.tensor.matmul(
                                            pz[:, s, :], ones_t[:], bias[:],
                                            start=False, stop=True)
                                if layer == 0:
                                    nc.scalar.activation(
                                        Tbf[:, bass.ts(c, 4), b, :], pz[:], relu)
                                else:
                                    tmp = work.tile([128, 4, H], F32, tag="tmp")
                                    nc.scalar.activation(tmp[:], pz[:], relu,
                                                         scale=STEP)
                                    nc.vector.tensor_add(
                                        Hsb[:, b, bass.ts(c, 4), :],
                                        Hsb[:, b, bass.ts(c, 4), :], tmp[:])
                                    if step == N_STEPS - 1:
                                        # final h: stream out as soon as ready.
                                        # One contiguous-per-partition DMA per
                                        # unit: software-DGE triggers cost
                                        # ~640ns of gpsimd engine time each.
                                        # The tail rides the HW sync queue
                                        # (idle once the last chunk loaded) so
                                        # the slow gpsimd drain isn't terminal.
                                        eng = nc.sync if c >= NCH - 2 else nc.gpsimd
                                        eng.dma_start(
                                            out[b, :, bass.ts(c, 4), :],
                                            Hsb[:, b, bass.ts(c, 4), :])
                                    else:
                                        nc.vector.tensor_copy(
                                            Hbf[:, bass.ts(c, 4), b, :],
                                            Hsb[:, b, bass.ts(c, 4), :])

                            flush_pending()
                            pending[0] = proj
            flush_pending()

    _split_multiwait(nc)
    return nc


_NC_CACHE = {}


def _get_nc(with_bias):
    if with_bias not in _NC_CACHE:
        _NC_CACHE[with_bias] = _build(with_bias)
    return _NC_CACHE[with_bias]


def _bf(a):
    return np.ascontiguousarray(a.astype(ml_dtypes.bfloat16))


def _prep_in_maps(x, adj, W_fc, b_fc, W1, b1, W2, b2):
    x = np.asarray(x, dtype=np.float32)
    adj = np.asarray(adj, dtype=np.float32)
    W_fc = np.asarray(W_fc, dtype=np.float32)
    b_fc = np.asarray(b_fc, dtype=np.float32)
    W1 = np.asarray(W1, dtype=np.float32)
    b1 = np.asarray(b1, dtype=np.float32)
    W2 = np.asarray(W2, dtype=np.float32)
    b2 = np.asarray(b2, dtype=np.float32)

    with_bias = bool(np.any(b_fc) or np.any(b1) or np.any(b2))
    x_folded = not with_bias

    # adjT chunked for the moving operand: [chunk, p, mt, n'] with
    # m = mt*128 + p (mt dim viewed as [mt2, 2] pairs for DoubleRow).
    adjT = np.ascontiguousarray(adj.T) * ADJ_SCALE
    adjm = (adjT.reshape(NT, 128, N).transpose(1, 0, 2)      # [p, mt, n]
            .reshape(128, NT, NCH, CHW).transpose(2, 0, 1, 3))  # [c, p, mt, n']
    adjm = np.ascontiguousarray(adjm.reshape(NCH, 128, NT // 2, 2, CHW)
                                .astype(ml_dtypes.float8_e4m3))

    w1h, w2h = W1 / ADJ_SCALE, W2 / ADJ_SCALE
    wfc_hi = W_fc.astype(ml_dtypes.bfloat16).astype(np.float32)
    wfc_lo = W_fc - wfc_hi
    wpack = np.zeros((128, 512), dtype=np.float32)
    wpack[:, 0:128] = w1h
    wpack[:, 128:256] = w2h
    wpack[0:IN_DIM, 256:384] = wfc_hi
    wpack[0:IN_DIM, 384:512] = wfc_lo
    wpack[IN_DIM:128, 256:384] = wfc_hi
    shared = {"adjm": adjm, "wpack": _bf(wpack)}
    if x_folded:
        wfc1 = (W_fc @ W1) / ADJ_SCALE
        wfc1s = np.zeros((128, H), dtype=np.float32)
        wfc1s[0:IN_DIM] = wfc1
        wfc1s[IN_DIM:128] = wfc1
        shared["wfc1s"] = _bf(wfc1s)
    if with_bias:
        shared.update({
            "b_fc": _bf(b_fc.reshape(1, H)),
            "b1": _bf(b1.reshape(1, H)),
            "b2": _bf(b2.reshape(1, H)),
            "ones": np.ones((1, H), dtype=ml_dtypes.bfloat16),
        })

    in_maps = []
    for cc in range(N_CORES):
        xs = x[cc * BL:(cc + 1) * BL]               # [BL, N, IN_DIM]
        xt = np.ascontiguousarray(xs.transpose(0, 2, 1))  # [BL, IN_DIM, N]
        xt_hi = xt.astype(ml_dtypes.bfloat16)
        xt_lo = _bf(xt - xt_hi.astype(np.float32))
        m = {**shared,
             "xt_hi": np.ascontiguousarray(xt_hi),
             "xt_lo": xt_lo}
        if x_folded:
            # [p, mt, bpair, bp*64+f] with b = 2*bpair + bp, m = mt*128+p
            x8 = (xs.reshape(2, 2, NT, 128, IN_DIM)
                  .transpose(3, 2, 0, 1, 4).reshape(128, NT, 2, 128))
            m["x8"] = np.ascontiguousarray(x8.astype(ml_dtypes.float8_e4m3))
        in_maps.append(m)
    return in_maps, with_bias


def gather(res):
    return np.concatenate(
        [np.asarray(res.results[c]["out"]).transpose(0, 2, 1, 3)
         .reshape(BL, N, H) for c in range(N_CORES)], axis=0)


def kernel(**inputs):
    in_maps, with_bias = _prep_in_maps(**inputs)
    nc = _get_nc(with_bias)
    res = run_bass_kernel_spmd(nc, in_maps, core_ids=list(range(N_CORES)))
    return gather(res)


def run_traced(**inputs):
    in_maps, with_bias = _prep_in_maps(**inputs)
    nc = _get_nc(with_bias)
    return run_bass_kernel_spmd(nc, in_maps, core_ids=list(range(N_CORES)),
                                trace=True)


# revision 42
# speedup vs baseline: 1.1955x; 1.1955x over previous
"""Graph-ODE (GCN message passing) Trainium2 kernel.

Problem: h0 = x @ W_fc + b_fc; 4 Euler steps of
  h <- h + 0.25 * relu(gcn2(relu(gcn1(h)))),  gcn(h) = (adj @ h) @ W + b
with B=32, N=4096, IN_DIM=64, H=128.

Strategy (8 NeuronCores, data-parallel over batch):
 - Each core owns 4 batches; adj (pre-transposed + tiled on host) and
   weights are replicated. No collectives.
 - Aggregation adj @ V runs with the ACTIVATION as the stationary
   operand (V slab [m,128h] per batch) and adjT as the moving operand
   ([m, 512n] chunks), in fp8-e4m3 DoubleRow (K=256/slab). The output
   lands PRE-TRANSPOSED [h, n] in PSUM, so the projection consumes it
   directly (stationary=aggT slab, moving=W) and emits z back in
   [n, h] node-major form — the form the next aggregation's stationary
   needs. No PE transposes anywhere (the old moving-V scheme burned
   ~1024 transposes + ACT drains per run).
 - adj is scaled by 4096 on the host so entries sit in e4m3 normal
   range; the scale folds back via W/4096 in the projection.
 - Step-0 layer-1 aggregates x directly (adj@(x@Wfc) = (adj@x)@Wfc with
   W_fc@W1 folded on the host); two batches share one stationary
   ([m, 2x64] columns) so the 64-wide features still halve that layer.
 - Projections for unit k are emitted between the aggregation chains of
   unit k+1 so the PE never waits on the PSUM->SBUF drain.
 - Euler state h stays fp32 in SBUF; h0 = x@W_fc uses a 3-term bf16
   hi/lo split since it dominates the output.
 - DMA: adjT streams as 2MB chunks split across the sync (HW-DGE) and
   gpsimd (software-DGE) queues; phase-0 x rides sync/scalar. Outputs
   leave per-unit as single contiguous-per-partition DMAs (h state and
   the out tensor are p-major; host transposes back) because software
   DGE triggers cost ~640ns of gpsimd engine time each. Tail outputs
   ride sync so the kernel doesn't end on a software-queue drain.
   Measured: ~955-970us HW exec (PE ~95% busy; the fp8-DoubleRow
   aggregation streams at its 1-row/cycle silicon rate, ~829us of the
   ~908us total PE work), 6.9e-5 relative error vs the fp32 reference.
   (Prior moving-V scheme with PE transposes: 1270us on the same HW —
   its strided moving-operand APs also held the PE ~20% under rate.)
"""
import sys

sys.path.insert(0, "/opt/trn_rl_repo")

import numpy as np
import ml_dtypes

import concourse.bass as bass
import concourse.mybir as mybir
import concourse.tile as tile
from concourse.bass_utils import run_bass_kernel_spmd

BF16 = mybir.dt.bfloat16
FP8 = mybir.dt.float8e4
F32 = mybir.dt.float32
ADJ_SCALE = 4096.0

B, N, IN_DIM, H = 32, 4096, 64, 128
N_CORES = 8
BL = B // N_CORES          # 4 batches per core
NT = N // 128              # 32 node tiles
NCH = 8                    # 512-wide n chunks
CHW = N // NCH             # 512
STEP = 0.25
N_STEPS = 4


def _split_multiwait(nc):
    """This walrus build accepts only ONE sync-wait command per engine
    instruction (incl. drains). Hoist extra waits onto preceding
    single-wait InstNoOps on the same engine."""
    import bass_rust
    for fn in nc.m.functions:
        for blk in fn.blocks:
            out = []
            for inst in blk.instructions:
                si = inst.sync_info
                if (si is not None and si.on_wait and len(si.on_wait) > 1
                        and type(inst).__name__ not in (
                            "InstTensorLoad", "InstTensorSave", "InstTrigger")):
                    waits = list(si.on_wait)
                    for w in waits[:-1]:
                        out.append(mybir.InstNoOp(
                            name=nc.get_next_instruction_name(),
                            engine=inst.engine, ins=[], outs=[],
                            sync_info=bass_rust.SyncInfo(
                                on_wait=[w], on_update=[]),
                        ))
                    inst.sync_info = bass_rust.SyncInfo(
                        on_wait=[waits[-1]], on_update=list(si.on_update))
                out.append(inst)
            blk.instructions = out


def _build(with_bias):
    nc = bass.Bass()
    x_folded = not with_bias

    # adjT chunked: [chunk, p, mt2, i, n'] with m=(2*mt2+i)*128+p,
    # n = chunk*512+n'; scaled by ADJ_SCALE, fp8.
    adjm = nc.dram_tensor("adjm", [NCH, 128, NT // 2, 2, CHW], FP8,
                          kind="ExternalInput")
    if x_folded:
        x8 = nc.dram_tensor("x8", [128, NT, 2, 128], FP8, kind="ExternalInput")
        wfc1s = nc.dram_tensor("wfc1s", [128, H], BF16, kind="ExternalInput")
    xt_hi = nc.dram_tensor("xt_hi", [BL, IN_DIM, N], BF16, kind="ExternalInput")
    xt_lo = nc.dram_tensor("xt_lo", [BL, IN_DIM, N], BF16, kind="ExternalInput")
    wpack = nc.dram_tensor("wpack", [128, 512], BF16, kind="ExternalInput")
    if with_bias:
        b_fc = nc.dram_tensor("b_fc", [1, H], BF16, kind="ExternalInput")
        b1 = nc.dram_tensor("b1", [1, H], BF16, kind="ExternalInput")
        b2 = nc.dram_tensor("b2", [1, H], BF16, kind="ExternalInput")
        ones = nc.dram_tensor("ones", [1, H], BF16, kind="ExternalInput")
    out = nc.dram_tensor("out", [BL, 128, NT, H], F32, kind="ExternalOutput")

    relu = mybir.ActivationFunctionType.Relu
    DR = mybir.MatmulPerfMode.DoubleRow

    with tile.TileContext(nc) as tc:
        with tc.tile_pool(name="res", bufs=1) as res, \
             tc.tile_pool(name="wgt", bufs=1) as wgt, \
             tc.tile_pool(name="xs", bufs=3) as xs, \
             tc.tile_pool(name="adjs", bufs=4) as adjs, \
             tc.tile_pool(name="work", bufs=3) as work, \
             tc.tile_pool(name="psA", bufs=4, space="PSUM") as psA, \
             tc.tile_pool(name="psZ", bufs=4, space="PSUM") as psZ:

            # --- resident state: fp8 activations in [p, mt, b, h]
            # node-major form (node m = mt*128 + p) to serve as matmul
            # stationaries; fp32 h in [p, b, mt, h] so each unit's final
            # output is one contiguous-per-partition DMA
            Hsb = res.tile([128, BL, NT, H], F32, tag="Hsb")
            Hbf = res.tile([128, NT, BL, H], FP8, tag="Hbf")
            Tbf = res.tile([128, NT, BL, H], FP8, tag="Tbf")

            # --- constants
            wpack_t = wgt.tile([128, 512], BF16, tag="wpack")
            nc.sync.dma_start(wpack_t[:], wpack[:])
            w1_t = wpack_t[:, 0:128]
            w2_t = wpack_t[:, 128:256]
            wfc_hi_t = wpack_t[0:IN_DIM, 256:384]
            wfc_lo_t = wpack_t[0:IN_DIM, 384:512]
            if x_folded:
                wfc1s_t = wgt.tile([128, H], BF16, tag="wfc1s")
                nc.sync.dma_start(wfc1s_t[:], wfc1s[:])
            if with_bias:
                bfc_t = wgt.tile([1, H], BF16, tag="bfc")
                b1_t = wgt.tile([1, H], BF16, tag="b1")
                b2_t = wgt.tile([1, H], BF16, tag="b2")
                ones_t = wgt.tile([1, H], BF16, tag="ones")
                nc.sync.dma_start(bfc_t[:], b_fc[:])
                nc.sync.dma_start(b1_t[:], b1[:])
                nc.sync.dma_start(b2_t[:], b2[:])
                nc.sync.dma_start(ones_t[:], ones[:])

            # --- phase 0: h0 = x @ W_fc (+ b_fc), 3-term bf16 hi/lo split.
            # One unit = one (batch, 512-col chunk): 4 nt tiles into one
            # PSUM bank, drained with one wide vector copy.
            def emit_p0_unit(b, c):
                xh = xs.tile([IN_DIM, CHW], BF16, tag="xh")
                xl = xs.tile([IN_DIM, CHW], BF16, tag="xl")
                nc.sync.dma_start(xh[:], xt_hi[b, :, bass.ts(c, CHW)])
                nc.scalar.dma_start(xl[:], xt_lo[b, :, bass.ts(c, CHW)])
                pz = psZ.tile([128, 4, H], F32, tag="pz")
                for j in range(4):
                    xhs = xh[:, bass.ts(j, 128)]
                    xls = xl[:, bass.ts(j, 128)]
                    nc.tensor.matmul(pz[:, j, :], xhs, wfc_hi_t,
                                     start=True, stop=False)
                    nc.tensor.matmul(pz[:, j, :], xls, wfc_hi_t,
                                     start=False, stop=False)
                    nc.tensor.matmul(pz[:, j, :], xhs, wfc_lo_t,
                                     start=False, stop=not with_bias)
                    if with_bias:
                        nc.tensor.matmul(pz[:, j, :], ones_t[:], bfc_t[:],
                                         start=False, stop=True)
                nc.vector.tensor_copy(Hsb[:, b, bass.ts(c, 4), :], pz[:])
                if not x_folded:
                    nc.scalar.activation(
                        Hbf[:, bass.ts(c, 4), b, :], pz[:],
                        mybir.ActivationFunctionType.Copy)

            # phase-0 units: a few emitted up front to cover the initial
            # adjT/x8 fills, the rest interleaved between x-layer units
            p0_iter = iter([(b, c) for b in range(BL) for c in range(NCH)])

            def emit_some_p0(n):
                for _ in range(n):
                    u = next(p0_iter, None)
                    if u is not None:
                        emit_p0_unit(*u)

            emit_some_p0(8 if x_folded else 32)

            if x_folded:
                X8t = res.tile([128, NT, 2, 128], FP8, tag="X8t")
                for c8 in range(4):
                    nc.gpsimd.dma_start(X8t[:, bass.ts(c8, 8), :, :],
                                        x8[:, bass.ts(c8, 8), :, :])

            # --- adjT chunk streaming (re-DMAed per aggregation pass);
            # each 2MB chunk split across the sync (HW-DGE) and gpsimd
            # (software-DGE) queues
            def load_chunk(c):
                ch = adjs.tile([128, NT // 2, 2, CHW], FP8, tag="chunk")
                nc.sync.dma_start(ch[:, 0:NT // 4, :, :], adjm[c, :, 0:NT // 4])
                nc.gpsimd.dma_start(ch[:, NT // 4:, :, :], adjm[c, :, NT // 4:])
                return ch

            # Deferred-projection queue: the proj/drain of unit k is
            # emitted after the aggregation chains of unit k+1 so the
            # PE never waits on the PSUM->SBUF drain.
            pending = [None]

            def flush_pending():
                if pending[0] is not None:
                    pending[0]()
                    pending[0] = None

            # --- step0/layer1 via x: adj@(x@Wfc) = (adj@x)@Wfc with
            # W_fc@W1 folded on the host. Stationary = x slab
            # [m, 2 batches x 64 feats] so 64-wide features halve it.
            if x_folded:
                for c in range(NCH):
                    ch = load_chunk(c)
                    for bpair in range(2):
                        pa = psA.tile([128, CHW], F32, tag="pa")
                        for mt2 in range(NT // 2):
                            nc.tensor.matmul(
                                pa[:], X8t[:, bass.ts(mt2, 2), bpair, :],
                                ch[:, mt2, :, :],
                                start=(mt2 == 0), stop=(mt2 == NT // 2 - 1),
                                perf_mode=DR)

                        def xproj(c=c, bpair=bpair, pa=pa):
                            ag = work.tile([128, CHW], BF16, tag="ag")
                            nc.vector.tensor_copy(ag[:], pa[:])
                            for bp in range(2):
                                b = 2 * bpair + bp
                                pz = psZ.tile([128, 4, H], F32, tag="pz")
                                for s in range(4):
                                    nc.tensor.matmul(
                                        pz[:, s, :],
                                        ag[bass.ds(64 * bp, 64),
                                           bass.ts(s, 128)],
                                        wfc1s_t[bass.ds(64 * bp, 64), :],
                                        start=True, stop=True)
                                nc.scalar.activation(
                                    Tbf[:, bass.ts(c, 4), b, :], pz[:], relu)

                        flush_pending()
                        pending[0] = xproj
                        emit_some_p0(2)
                emit_some_p0(32)

            # --- 4 Euler steps x 2 GCN layers ---
            for step in range(N_STEPS):
                for layer in range(2):
                    if x_folded and step == 0 and layer == 0:
                        continue
                    V = Hbf if layer == 0 else Tbf
                    W = w1_t if layer == 0 else w2_t
                    bias = None if not with_bias else (b1_t if layer == 0 else b2_t)
                    for c in range(NCH):
                        ch = load_chunk(c)
                        for b in range(BL):
                            pa = psA.tile([128, CHW], F32, tag="pa")
                            for mt2 in range(NT // 2):
                                nc.tensor.matmul(
                                    pa[:], V[:, bass.ts(mt2, 2), b, :],
                                    ch[:, mt2, :, :],
                                    start=(mt2 == 0), stop=(mt2 == NT // 2 - 1),
                                    perf_mode=DR)

                            def proj(c=c, b=b, pa=pa, W=W, bias=bias,
                                     layer=layer, step=step):
                                ag = work.tile([128, CHW], BF16, tag="ag")
                                nc.vector.tensor_copy(ag[:], pa[:])
                                pz = psZ.tile([128, 4, H], F32, tag="pz")
                                for s in range(4):
                                    nc.tensor.matmul(
                                        pz[:, s, :], ag[:, bass.ts(s, 128)], W,
                                        start=True, stop=bias is None)
                                    if bias is not None:
                                        nc.tensor.matmul(
                                            pz[:, s, :], ones_t[:], bias[:],
                                            start=False, stop=True)
                                if layer == 0:
                                    nc.scalar.activation(
                                        Tbf[:, bass.ts(c, 4), b, :], pz[:], relu)
                                else:
                                    tmp = work.tile([128, 4, H], F32, tag="tmp")
                                    nc.scalar.activation(tmp[:], pz[:], relu,
                                                         scale=STEP)
                                    nc.vector.tensor_add(
                                        Hsb[:, b, bass.ts(c, 4), :],
                                        Hsb[:, b, bass.ts(c, 4), :], tmp[:])
                                    if step == N_STEPS - 1:
                                        # final h: stream out as soon as ready.
                                        # One contiguous-per-partition DMA per
                                        # unit: software-DGE triggers cost
                                        # ~640ns of gpsimd engine time each.
                                        # The tail rides the HW sync queue
                                        # (idle once the last chunk loaded) so
                                        # the slow gpsimd drain isn't terminal.
                                        eng = nc.sync if c >= NCH - 2 else nc.gpsimd
                                        eng.dma_start(
                                            out[b, :, bass.ts(c, 4), :],
                                            Hsb[:, b, bass.ts(c, 4), :])
                                    else:
                                        nc.vector.tensor_copy(
                                            Hbf[:, bass.ts(c, 4), b, :],
                                            Hsb[:, b, bass.ts(c, 4), :])

                            flush_pending()
                            pending[0] = proj
            flush_pending()

    _split_multiwait(nc)
    return nc


_NC_CACHE = {}


def _get_nc(with_bias):
    if with_bias not in _NC_CACHE:
        _NC_CACHE[with_bias] = _build(with_bias)
    return _NC_CACHE[with_bias]


def _bf(a):
    return np.ascontiguousarray(a.astype(ml_dtypes.bfloat16))


def _prep_in_maps(x, adj, W_fc, b_fc, W1, b1, W2, b2):
    x = np.asarray(x, dtype=np.float32)
    adj = np.asarray(adj, dtype=np.float32)
    W_fc = np.asarray(W_fc, dtype=np.float32)
    b_fc = np.asarray(b_fc, dtype=np.float32)
    W1 = np.asarray(W1, dtype=np.float32)
    b1 = np.asarray(b1, dtype=np.float32)
    W2 = np.asarray(W2, dtype=np.float32)
    b2 = np.asarray(b2, dtype=np.float32)

    with_bias = bool(np.any(b_fc) or np.any(b1) or np.any(b2))
    x_folded = not with_bias

    # adjT chunked for the moving operand: [chunk, p, mt, n'] with
    # m = mt*128 + p (mt dim viewed as [mt2, 2] pairs for DoubleRow).
    adjT = np.ascontiguousarray(adj.T) * ADJ_SCALE
    adjm = (adjT.reshape(NT, 128, N).transpose(1, 0, 2)      # [p, mt, n]
            .reshape(128, NT, NCH, CHW).transpose(2, 0, 1, 3))  # [c, p, mt, n']
    adjm = np.ascontiguousarray(adjm.reshape(NCH, 128, NT // 2, 2, CHW)
                                .astype(ml_dtypes.float8_e4m3))

    w1h, w2h = W1 / ADJ_SCALE, W2 / ADJ_SCALE
    wfc_hi = W_fc.astype(ml_dtypes.bfloat16).astype(np.float32)
    wfc_lo = W_fc - wfc_hi
    wpack = np.zeros((128, 512), dtype=np.float32)
    wpack[:, 0:128] = w1h
    wpack[:, 128:256] = w2h
    wpack[0:IN_DIM, 256:384] = wfc_hi
    wpack[0:IN_DIM, 384:512] = wfc_lo
    shared = {"adjm": adjm, "wpack": _bf(wpack)}
    if x_folded:
        wfc1 = (W_fc @ W1) / ADJ_SCALE
        wfc1s = np.zeros((128, H), dtype=np.float32)
        wfc1s[0:IN_DIM] = wfc1
        wfc1s[IN_DIM:128] = wfc1
        shared["wfc1s"] = _bf(wfc1s)
    if with_bias:
        shared.update({
            "b_fc": _bf(b_fc.reshape(1, H)),
            "b1": _bf(b1.reshape(1, H)),
            "b2": _bf(b2.reshape(1, H)),
            "ones": np.ones((1, H), dtype=ml_dtypes.bfloat16),
        })

    in_maps = []
    for cc in range(N_CORES):
        xs = x[cc * BL:(cc + 1) * BL]               # [BL, N, IN_DIM]
        xt = np.ascontiguousarray(xs.transpose(0, 2, 1))  # [BL, IN_DIM, N]
        xt_hi = xt.astype(ml_dtypes.bfloat16)
        xt_lo = _bf(xt - xt_hi.astype(np.float32))
        m = {**shared,
             "xt_hi": np.ascontiguousarray(xt_hi),
             "xt_lo": xt_lo}
        if x_folded:
            # [p, mt, bpair, bp*64+f] with b = 2*bpair + bp, m = mt*128+p
            x8 = (xs.reshape(2, 2, NT, 128, IN_DIM)
                  .transpose(3, 2, 0, 1, 4).reshape(128, NT, 2, 128))
            m["x8"] = np.ascontiguousarray(x8.astype(ml_dtypes.float8_e4m3))
        in_maps.append(m)
    return in_maps, with_bias


def gather(res):
    return np.concatenate(
        [np.asarray(res.results[c]["out"]).transpose(0, 2, 1, 3)
         .reshape(BL, N, H) for c in range(N_CORES)], axis=0)


def kernel(**inputs):
    in_maps, with_bias = _prep_in_maps(**inputs)
    nc = _get_nc(with_bias)
    res = run_bass_kernel_spmd(nc, in_maps, core_ids=list(range(N_CORES)))
    return gather(res)


def run_traced(**inputs):
    in_maps, with_bias = _prep_in_maps(**inputs)
    nc = _get_nc(with_bias)
    return run_bass_kernel_spmd(nc, in_maps, core_ids=list(range(N_CORES)),
                                trace=True)


# revision 43
# speedup vs baseline: 1.2033x; 1.0065x over previous
"""Graph-ODE (GCN message passing) Trainium2 kernel.

Problem: h0 = x @ W_fc + b_fc; 4 Euler steps of
  h <- h + 0.25 * relu(gcn2(relu(gcn1(h)))),  gcn(h) = (adj @ h) @ W + b
with B=32, N=4096, IN_DIM=64, H=128.

Strategy (8 NeuronCores, data-parallel over batch):
 - Each core owns 4 batches; adj (pre-transposed + tiled on host) and
   weights are replicated. No collectives.
 - Aggregation adj @ V runs with the ACTIVATION as the stationary
   operand (V slab [m,128h] per batch) and adjT as the moving operand
   ([m, 512n] chunks), in fp8-e4m3 DoubleRow (K=256/slab). The output
   lands PRE-TRANSPOSED [h, n] in PSUM, so the projection consumes it
   directly (stationary=aggT slab, moving=W) and emits z back in
   [n, h] node-major form — the form the next aggregation's stationary
   needs. No PE transposes anywhere (the old moving-V scheme burned
   ~1024 transposes + ACT drains per run).
 - adj is scaled by 4096 on the host so entries sit in e4m3 normal
   range; the scale folds back via W/4096 in the projection.
 - Step-0 layer-1 aggregates x directly (adj@(x@Wfc) = (adj@x)@Wfc with
   W_fc@W1 folded on the host); two batches share one stationary
   ([m, 2x64] columns) so the 64-wide features still halve that layer.
 - Projections for unit k are emitted between the aggregation chains of
   unit k+1 so the PE never waits on the PSUM->SBUF drain.
 - Euler state h stays fp32 in SBUF; h0 = x@W_fc uses a 3-term bf16
   hi/lo split since it dominates the output.
 - DMA: adjT streams as 2MB chunks split across the sync (HW-DGE) and
   gpsimd (software-DGE) queues; phase-0 x rides sync/scalar. Outputs
   leave per-unit as single contiguous-per-partition DMAs (h state and
   the out tensor are p-major; host transposes back) because software
   DGE triggers cost ~640ns of gpsimd engine time each. Tail outputs
   ride sync so the kernel doesn't end on a software-queue drain.
   Measured: ~955-970us HW exec (PE ~95% busy; the fp8-DoubleRow
   aggregation streams at its 1-row/cycle silicon rate, ~829us of the
   ~908us total PE work), 6.9e-5 relative error vs the fp32 reference.
   (Prior moving-V scheme with PE transposes: 1270us on the same HW —
   its strided moving-operand APs also held the PE ~20% under rate.)
"""
import sys

sys.path.insert(0, "/opt/trn_rl_repo")

import numpy as np
import ml_dtypes

import concourse.bass as bass
import concourse.mybir as mybir
import concourse.tile as tile
from concourse.bass_utils import run_bass_kernel_spmd

BF16 = mybir.dt.bfloat16
FP8 = mybir.dt.float8e4
F32 = mybir.dt.float32
ADJ_SCALE = 4096.0

B, N, IN_DIM, H = 32, 4096, 64, 128
N_CORES = 8
BL = B // N_CORES          # 4 batches per core
NT = N // 128              # 32 node tiles
NCH = 8                    # 512-wide n chunks
CHW = N // NCH             # 512
STEP = 0.25
N_STEPS = 4


def _split_multiwait(nc):
    """This walrus build accepts only ONE sync-wait command per engine
    instruction (incl. drains). Hoist extra waits onto preceding
    single-wait InstNoOps on the same engine."""
    import bass_rust
    for fn in nc.m.functions:
        for blk in fn.blocks:
            out = []
            for inst in blk.instructions:
                si = inst.sync_info
                if (si is not None and si.on_wait and len(si.on_wait) > 1
                        and type(inst).__name__ not in (
                            "InstTensorLoad", "InstTensorSave", "InstTrigger")):
                    waits = list(si.on_wait)
                    for w in waits[:-1]:
                        out.append(mybir.InstNoOp(
                            name=nc.get_next_instruction_name(),
                            engine=inst.engine, ins=[], outs=[],
                            sync_info=bass_rust.SyncInfo(
                                on_wait=[w], on_update=[]),
                        ))
                    inst.sync_info = bass_rust.SyncInfo(
                        on_wait=[waits[-1]], on_update=list(si.on_update))
                out.append(inst)
            blk.instructions = out


def _build(with_bias):
    nc = bass.Bass()
    x_folded = not with_bias

    # adjT chunked: [chunk, p, mt2, i, n'] with m=(2*mt2+i)*128+p,
    # n = chunk*512+n'; scaled by ADJ_SCALE, fp8.
    adjm = nc.dram_tensor("adjm", [NCH, 128, NT // 2, 2, CHW], FP8,
                          kind="ExternalInput")
    if x_folded:
        x8 = nc.dram_tensor("x8", [128, NT, 2, 128], FP8, kind="ExternalInput")
        wfc1s = nc.dram_tensor("wfc1s", [128, H], BF16, kind="ExternalInput")
    xt_hi = nc.dram_tensor("xt_hi", [BL, IN_DIM, N], BF16, kind="ExternalInput")
    xt_lo = nc.dram_tensor("xt_lo", [BL, IN_DIM, N], BF16, kind="ExternalInput")
    wpack = nc.dram_tensor("wpack", [128, 512], BF16, kind="ExternalInput")
    if with_bias:
        b_fc = nc.dram_tensor("b_fc", [1, H], BF16, kind="ExternalInput")
        b1 = nc.dram_tensor("b1", [1, H], BF16, kind="ExternalInput")
        b2 = nc.dram_tensor("b2", [1, H], BF16, kind="ExternalInput")
        ones = nc.dram_tensor("ones", [1, H], BF16, kind="ExternalInput")
    out = nc.dram_tensor("out", [BL, 128, NT, H], F32, kind="ExternalOutput")

    relu = mybir.ActivationFunctionType.Relu
    DR = mybir.MatmulPerfMode.DoubleRow

    with tile.TileContext(nc) as tc:
        with tc.tile_pool(name="res", bufs=1) as res, \
             tc.tile_pool(name="wgt", bufs=1) as wgt, \
             tc.tile_pool(name="xs", bufs=4) as xs, \
             tc.tile_pool(name="adjs", bufs=4) as adjs, \
             tc.tile_pool(name="work", bufs=3) as work, \
             tc.tile_pool(name="psA", bufs=4, space="PSUM") as psA, \
             tc.tile_pool(name="psZ", bufs=4, space="PSUM") as psZ:

            # --- resident state: fp8 activations in [p, mt, b, h]
            # node-major form (node m = mt*128 + p) to serve as matmul
            # stationaries; fp32 h in [p, b, mt, h] so each unit's final
            # output is one contiguous-per-partition DMA
            Hsb = res.tile([128, BL, NT, H], F32, tag="Hsb")
            Hbf = res.tile([128, NT, BL, H], FP8, tag="Hbf")
            Tbf = res.tile([128, NT, BL, H], FP8, tag="Tbf")

            # --- constants
            wpack_t = wgt.tile([128, 512], BF16, tag="wpack")
            nc.sync.dma_start(wpack_t[:], wpack[:])
            w1_t = wpack_t[:, 0:128]
            w2_t = wpack_t[:, 128:256]
            wfc_hi_t = wpack_t[0:IN_DIM, 256:384]
            wfc_lo_t = wpack_t[0:IN_DIM, 384:512]
            if x_folded:
                wfc1s_t = wgt.tile([128, H], BF16, tag="wfc1s")
                nc.sync.dma_start(wfc1s_t[:], wfc1s[:])
            if with_bias:
                bfc_t = wgt.tile([1, H], BF16, tag="bfc")
                b1_t = wgt.tile([1, H], BF16, tag="b1")
                b2_t = wgt.tile([1, H], BF16, tag="b2")
                ones_t = wgt.tile([1, H], BF16, tag="ones")
                nc.sync.dma_start(bfc_t[:], b_fc[:])
                nc.sync.dma_start(b1_t[:], b1[:])
                nc.sync.dma_start(b2_t[:], b2[:])
                nc.sync.dma_start(ones_t[:], ones[:])

            # --- phase 0: h0 = x @ W_fc (+ b_fc), 3-term bf16 hi/lo split.
            # One unit = one (batch, 512-col chunk): 4 nt tiles into one
            # PSUM bank, drained with one wide vector copy.
            def emit_p0_unit(b, c):
                xh = xs.tile([IN_DIM, CHW], BF16, tag="xh")
                xl = xs.tile([IN_DIM, CHW], BF16, tag="xl")
                nc.sync.dma_start(xh[:], xt_hi[b, :, bass.ts(c, CHW)])
                nc.scalar.dma_start(xl[:], xt_lo[b, :, bass.ts(c, CHW)])
                pz = psZ.tile([128, 4, H], F32, tag="pz")
                for j in range(4):
                    xhs = xh[:, bass.ts(j, 128)]
                    xls = xl[:, bass.ts(j, 128)]
                    nc.tensor.matmul(pz[:, j, :], xhs, wfc_hi_t,
                                     start=True, stop=False)
                    nc.tensor.matmul(pz[:, j, :], xls, wfc_hi_t,
                                     start=False, stop=False)
                    nc.tensor.matmul(pz[:, j, :], xhs, wfc_lo_t,
                                     start=False, stop=not with_bias)
                    if with_bias:
                        nc.tensor.matmul(pz[:, j, :], ones_t[:], bfc_t[:],
                                         start=False, stop=True)
                nc.vector.tensor_copy(Hsb[:, b, bass.ts(c, 4), :], pz[:])
                if not x_folded:
                    nc.scalar.activation(
                        Hbf[:, bass.ts(c, 4), b, :], pz[:],
                        mybir.ActivationFunctionType.Copy)

            # phase-0 units: a few emitted up front to cover the initial
            # adjT/x8 fills, the rest interleaved between x-layer units
            p0_iter = iter([(b, c) for b in range(BL) for c in range(NCH)])

            def emit_some_p0(n):
                for _ in range(n):
                    u = next(p0_iter, None)
                    if u is not None:
                        emit_p0_unit(*u)

            emit_some_p0(10 if x_folded else 32)

            if x_folded:
                X8t = res.tile([128, NT, 2, 128], FP8, tag="X8t")
                for c8 in range(4):
                    nc.gpsimd.dma_start(X8t[:, bass.ts(c8, 8), :, :],
                                        x8[:, bass.ts(c8, 8), :, :])

            # --- adjT chunk streaming (re-DMAed per aggregation pass);
            # each 2MB chunk split across the sync (HW-DGE) and gpsimd
            # (software-DGE) queues
            def load_chunk(c):
                ch = adjs.tile([128, NT // 2, 2, CHW], FP8, tag="chunk")
                nc.sync.dma_start(ch[:, 0:NT // 4, :, :], adjm[c, :, 0:NT // 4])
                nc.gpsimd.dma_start(ch[:, NT // 4:, :, :], adjm[c, :, NT // 4:])
                return ch

            # Deferred-projection queue: the proj/drain of unit k is
            # emitted after the aggregation chains of unit k+1 so the
            # PE never waits on the PSUM->SBUF drain.
            pending = [None]

            def flush_pending():
                if pending[0] is not None:
                    pending[0]()
                    pending[0] = None

            # --- step0/layer1 via x: adj@(x@Wfc) = (adj@x)@Wfc with
            # W_fc@W1 folded on the host. Stationary = x slab
            # [m, 2 batches x 64 feats] so 64-wide features halve it.
            if x_folded:
                for c in range(NCH):
                    ch = load_chunk(c)
                    for bpair in range(2):
                        pa = psA.tile([128, CHW], F32, tag="pa")
                        for mt2 in range(NT // 2):
                            nc.tensor.matmul(
                                pa[:], X8t[:, bass.ts(mt2, 2), bpair, :],
                                ch[:, mt2, :, :],
                                start=(mt2 == 0), stop=(mt2 == NT // 2 - 1),
                                perf_mode=DR)

                        def xproj(c=c, bpair=bpair, pa=pa):
                            ag = work.tile([128, CHW], BF16, tag="ag")
                            nc.vector.tensor_copy(ag[:], pa[:])
                            for bp in range(2):
                                b = 2 * bpair + bp
                                pz = psZ.tile([128, 4, H], F32, tag="pz")
                                for s in range(4):
                                    nc.tensor.matmul(
                                        pz[:, s, :],
                                        ag[bass.ds(64 * bp, 64),
                                           bass.ts(s, 128)],
                                        wfc1s_t[bass.ds(64 * bp, 64), :],
                                        start=True, stop=True)
                                nc.scalar.activation(
                                    Tbf[:, bass.ts(c, 4), b, :], pz[:], relu)

                        flush_pending()
                        pending[0] = xproj
                        emit_some_p0(2)
                emit_some_p0(32)

            # --- 4 Euler steps x 2 GCN layers ---
            for step in range(N_STEPS):
                for layer in range(2):
                    if x_folded and step == 0 and layer == 0:
                        continue
                    V = Hbf if layer == 0 else Tbf
                    W = w1_t if layer == 0 else w2_t
                    bias = None if not with_bias else (b1_t if layer == 0 else b2_t)
                    for c in range(NCH):
                        ch = load_chunk(c)
                        for b in range(BL):
                            pa = psA.tile([128, CHW], F32, tag="pa")
                            for mt2 in range(NT // 2):
                                nc.tensor.matmul(
                                    pa[:], V[:, bass.ts(mt2, 2), b, :],
                                    ch[:, mt2, :, :],
                                    start=(mt2 == 0), stop=(mt2 == NT // 2 - 1),
                                    perf_mode=DR)

                            def proj(c=c, b=b, pa=pa, W=W, bias=bias,
                                     layer=layer, step=step):
                                ag = work.tile([128, CHW], BF16, tag="ag")
                                nc.vector.tensor_copy(ag[:], pa[:])
                                pz = psZ.tile([128, 4, H], F32, tag="pz")
                                for s in range(4):
                                    nc.tensor.matmul(
                                        pz[:, s, :], ag[:, bass.ts(s, 128)], W,
                                        start=True, stop=bias is None)
                                    if bias is not None:
                                        nc.tensor.matmul(
                                            pz[:, s, :], ones_t[:], bias[:],
                                            start=False, stop=True)
                                if layer == 0:
                                    nc.scalar.activation(
                                        Tbf[:, bass.ts(c, 4), b, :], pz[:], relu)
                                else:
                                    tmp = work.tile([128, 4, H], F32, tag="tmp")
                                    nc.scalar.activation(tmp[:], pz[:], relu,
                                                         scale=STEP)
                                    nc.vector.tensor_add(
                                        Hsb[:, b, bass.ts(c, 4), :],
                                        Hsb[:, b, bass.ts(c, 4), :], tmp[:])
                                    if step == N_STEPS - 1:
                                        # final h: stream out as soon as ready.
                                        # One contiguous-per-partition DMA per
                                        # unit: software-DGE triggers cost
                                        # ~640ns of gpsimd engine time each.
                                        # The tail rides the HW sync queue
                                        # (idle once the last chunk loaded) so
                                        # the slow gpsimd drain isn't terminal.
                                        eng = nc.sync if c == NCH - 1 else nc.gpsimd
                                        eng.dma_start(
                                            out[b, :, bass.ts(c, 4), :],
                                            Hsb[:, b, bass.ts(c, 4), :])
                                    else:
                                        nc.vector.tensor_copy(
                                            Hbf[:, bass.ts(c, 4), b, :],
                                            Hsb[:, b, bass.ts(c, 4), :])

                            flush_pending()
                            pending[0] = proj
            flush_pending()

    _split_multiwait(nc)
    return nc


_NC_CACHE = {}


def _get_nc(with_bias):
    if with_bias not in _NC_CACHE:
        _NC_CACHE[with_bias] = _build(with_bias)
    return _NC_CACHE[with_bias]


def _bf(a):
    return np.ascontiguousarray(a.astype(ml_dtypes.bfloat16))


def _prep_in_maps(x, adj, W_fc, b_fc, W1, b1, W2, b2):
    x = np.asarray(x, dtype=np.float32)
    adj = np.asarray(adj, dtype=np.float32)
    W_fc = np.asarray(W_fc, dtype=np.float32)
    b_fc = np.asarray(b_fc, dtype=np.float32)
    W1 = np.asarray(W1, dtype=np.float32)
    b1 = np.asarray(b1, dtype=np.float32)
    W2 = np.asarray(W2, dtype=np.float32)
    b2 = np.asarray(b2, dtype=np.float32)

    with_bias = bool(np.any(b_fc) or np.any(b1) or np.any(b2))
    x_folded = not with_bias

    # adjT chunked for the moving operand: [chunk, p, mt, n'] with
    # m = mt*128 + p (mt dim viewed as [mt2, 2] pairs for DoubleRow).
    adjT = np.ascontiguousarray(adj.T) * ADJ_SCALE
    adjm = (adjT.reshape(NT, 128, N).transpose(1, 0, 2)      # [p, mt, n]
            .reshape(128, NT, NCH, CHW).transpose(2, 0, 1, 3))  # [c, p, mt, n']
    adjm = np.ascontiguousarray(adjm.reshape(NCH, 128, NT // 2, 2, CHW)
                                .astype(ml_dtypes.float8_e4m3))

    w1h, w2h = W1 / ADJ_SCALE, W2 / ADJ_SCALE
    wfc_hi = W_fc.astype(ml_dtypes.bfloat16).astype(np.float32)
    wfc_lo = W_fc - wfc_hi
    wpack = np.zeros((128, 512), dtype=np.float32)
    wpack[:, 0:128] = w1h
    wpack[:, 128:256] = w2h
    wpack[0:IN_DIM, 256:384] = wfc_hi
    wpack[0:IN_DIM, 384:512] = wfc_lo
    shared = {"adjm": adjm, "wpack": _bf(wpack)}
    if x_folded:
        wfc1 = (W_fc @ W1) / ADJ_SCALE
        wfc1s = np.zeros((128, H), dtype=np.float32)
        wfc1s[0:IN_DIM] = wfc1
        wfc1s[IN_DIM:128] = wfc1
        shared["wfc1s"] = _bf(wfc1s)
    if with_bias:
        shared.update({
            "b_fc": _bf(b_fc.reshape(1, H)),
            "b1": _bf(b1.reshape(1, H)),
            "b2": _bf(b2.reshape(1, H)),
            "ones": np.ones((1, H), dtype=ml_dtypes.bfloat16),
        })

    in_maps = []
    for cc in range(N_CORES):
        xs = x[cc * BL:(cc + 1) * BL]               # [BL, N, IN_DIM]
        xt = np.ascontiguousarray(xs.transpose(0, 2, 1))  # [BL, IN_DIM, N]
        xt_hi = xt.astype(ml_dtypes.bfloat16)
        xt_lo = _bf(xt - xt_hi.astype(np.float32))
        m = {**shared,
             "xt_hi": np.ascontiguousarray(xt_hi),
             "xt_lo": xt_lo}
        if x_folded:
            # [p, mt, bpair, bp*64+f] with b = 2*bpair + bp, m = mt*128+p
            x8 = (xs.reshape(2, 2, NT, 128, IN_DIM)
                  .transpose(3, 2, 0, 1, 4).reshape(128, NT, 2, 128))
            m["x8"] = np.ascontiguousarray(x8.astype(ml_dtypes.float8_e4m3))
        in_maps.append(m)
    return in_maps, with_bias


def gather(res):
    return np.concatenate(
        [np.asarray(res.results[c]["out"]).transpose(0, 2, 1, 3)
         .reshape(BL, N, H) for c in range(N_CORES)], axis=0)


def kernel(**inputs):
    in_maps, with_bias = _prep_in_maps(**inputs)
    nc = _get_nc(with_bias)
    res = run_bass_kernel_spmd(nc, in_maps, core_ids=list(range(N_CORES)))
    return gather(res)


def run_traced(**inputs):
    in_maps, with_bias = _prep_in_maps(**inputs)
    nc = _get_nc(with_bias)
    return run_bass_kernel_spmd(nc, in_maps, core_ids=list(range(N_CORES)),
                                trace=True)
